# revision 34
# baseline (speedup 1.0000x reference)
"""Trainium2 Bass kernel for nn_Discriminator: MLP + sort-based minibatch
discrimination with gpsimd local_scatter un-permutation. Self-contained."""
import numpy as np
import ml_dtypes

N = 2048
NROWS = 4
NCOLS = 512


def stages(n=None):
    if n is None:
        n = N
    out = []
    p = 1
    while p < n:
        k = p
        while k >= 1:
            lefts = []
            j = k % p
            while j <= n - 1 - k:
                for i in range(0, min(k, n - j - k)):
                    x = i + j
                    if (x // (2 * p)) == ((x + k) // (2 * p)):
                        lefts.append(x)
                j += 2 * k
            out.append((p, k, np.array(sorted(lefts), dtype=np.int64)))
            k //= 2
        p *= 2
    return out


def runs_of(xs):
    """Compress sorted ints into <=3-level pattern (start, L, s1, c1, s2, c2)."""
    xs = np.asarray(xs)
    if len(xs) == 0:
        return None
    breaks = np.where(np.diff(xs) != 1)[0]
    starts_i = np.concatenate([[0], breaks + 1])
    ends_i = np.concatenate([breaks, [len(xs) - 1]])
    run_starts = xs[starts_i]
    run_lens = ends_i - starts_i + 1
    if not np.all(run_lens == run_lens[0]):
        return None
    L = int(run_lens[0])
    if len(run_starts) == 1:
        return (int(run_starts[0]), L, 0, 1, 0, 1)
    d = np.diff(run_starts)
    if np.all(d == d[0]):
        return (int(run_starts[0]), L, int(d[0]), len(run_starts), 0, 1)
    s1 = d[0]
    c1 = 1
    while c1 < len(d) and d[c1 - 1] == s1:
        c1 += 1
    group = c1
    if len(run_starts) % group != 0:
        return None
    rs = run_starts.reshape(-1, group)
    inner = np.diff(rs, axis=1)
    starts2 = rs[:, 0]
    d2 = np.diff(starts2)
    if inner.size and not np.all(inner == s1):
        return None
    if len(d2) and not np.all(d2 == d2[0]):
        return None
    return (int(run_starts[0]), L, int(s1), group,
            int(d2[0]) if len(d2) else 0, len(starts2))


def emit_ops():
    """Returns list of (p, k, [ops]); op = (r0, nrows, drow, colpat, colB0)."""
    all_stages = []
    for (p, k, lefts) in stages():
        ops = []
        rows = lefts // NCOLS
        cols = lefts % NCOLS
        drows = (lefts + k) // NCOLS - rows
        for dr in np.unique(drows):
            sel = drows == dr
            rset = np.unique(rows[sel])
            cset = np.unique(cols[sel])
            assert sel.sum() == len(rset) * len(cset), (p, k, dr)
            for r in rset:
                cc = np.sort(cols[sel & (rows == r)])
                assert np.array_equal(cc, cset), (p, k, dr, r)
            colpat = runs_of(cset)
            assert colpat is not None, (p, k, dr, cset[:20])
            rpat = runs_of(rset)
            assert rpat is not None, (p, k, dr, rset)
            (r0, Lr, sr1, cr1, sr2, cr2) = rpat
            assert sr2 == 0 and cr2 == 1, (p, k, dr, rpat)
            colB0 = int((cset[0] + k) % NCOLS)
            for g in range(cr1):
                rstart = r0 + g * sr1
                ops.append((int(rstart), int(Lr), int(dr), colpat, colB0))
        all_stages.append((p, int(k), ops))
    return all_stages


def _row_chunks(a_base, b_base, nr):
    allowed = {0: 4, 1: 1, 2: 2, 3: 1}
    out = []
    off = 0
    while off < nr:
        c = min(allowed[(a_base + off) % 4], allowed[(b_base + off) % 4], nr - off)
        out.append((off, c))
        off += c
    return out


def legalize(all_stages):
    out = []
    for (p, k, ops) in all_stages:
        nops = []
        for (r0, nr, dr, colpat, colB0) in ops:
            for (off, c) in _row_chunks(r0, r0 + dr, nr):
                nops.append((r0 + off, c, dr, colpat, colB0))
        out.append((p, k, nops))
    return out


def colpat_idx(colpat):
    (c0, L, s1, c1, s2, c2) = colpat
    return (c0 + np.arange(c2)[:, None, None] * s2
            + np.arange(c1)[None, :, None] * s1
            + np.arange(L)[None, None, :]).ravel()


def runs_multi(xs, max_groups=6):
    xs = np.asarray(xs)
    if len(xs) == 0:
        return []
    r = runs_of(xs)
    if r is not None:
        return [r]
    breaks = np.where(np.diff(xs) != 1)[0]
    starts_i = np.concatenate([[0], breaks + 1])
    ends_i = np.concatenate([breaks, [len(xs) - 1]])
    run_starts = xs[starts_i]
    run_lens = ends_i - starts_i + 1
    out = []
    for L in np.unique(run_lens):
        sel = run_lens == L
        rs = run_starts[sel]
        d = np.diff(rs)
        if len(d) == 0 or np.all(d == d[0]):
            out.append((int(rs[0]), int(L), int(d[0]) if len(d) else 0,
                        len(rs), 0, 1))
        else:
            for s in rs:
                out.append((int(s), int(L), 0, 1, 0, 1))
    return out


def emit_pingpong():
    """cp op = (r0, nr, pat, old): old=True -> the cell was untouched in the
    previous stage too, so it can be copied from the 2-stages-old rotation
    buffer (dependency jumps a stage back; copy leaves the critical chain)."""
    out = []
    prev_touched = np.ones((NROWS, NCOLS), dtype=bool)
    for (p, k, ops) in legalize(emit_ops()):
        touched = np.zeros((NROWS, NCOLS), dtype=bool)
        for (r0, nr, dr, colpat, colB0) in ops:
            ia = colpat_idx(colpat)
            ib = ia + (colB0 - colpat[0])
            for rr in range(r0, r0 + nr):
                touched[rr, ia] = True
                touched[rr + dr, ib] = True
        cp_ops = []
        for old in (False, True):
            need = (~touched) & (prev_touched if not old else ~prev_touched)
            r = 0
            while r < NROWS:
                mask = need[r]
                r2 = r + 1
                while r2 < NROWS and np.array_equal(need[r2], mask):
                    r2 += 1
                cols = np.where(mask)[0]
                if len(cols):
                    for pat in runs_multi(cols):
                        off = 0
                        nr_ = r2 - r
                        allowed = {0: 4, 1: 1, 2: 2, 3: 1}
                        while off < nr_:
                            c = min(allowed[(r + off) % 4], nr_ - off)
                            cp_ops.append((r + off, c, pat, old))
                            off += c
                r = r2
        prev_touched = touched
        out.append((p, k, ops, cp_ops))
    return out


def _split_colpat(colpat, max_free=288):
    (c0, L, s1, c1, s2, c2) = colpat
    free = L * c1 * c2
    if free <= max_free:
        return [(0, colpat)]
    if c2 > 1:
        h = c2 // 2
        a = (c0, L, s1, c1, s2, h)
        b = (c0 + h * s2, L, s1, c1, s2, c2 - h)
        return [(d, p) for d0, pp_ in [(0, a), (h * s2, b)]
                for d, p in [(d0 + dd, p2) for dd, p2 in _split_colpat(
                    (pp_[0], pp_[1], pp_[2], pp_[3], pp_[4], pp_[5]), max_free)]]
    if c1 > 1:
        h = c1 // 2
        a = (c0, L, s1, h, 0, 1)
        b = (c0 + h * s1, L, s1, c1 - h, 0, 1)
        out = []
        for base, pat in [(0, a), (h * s1, b)]:
            out.extend(_split_colpat(pat, max_free))
        return out
    h = L // 2
    a = (c0, h, 0, 1, 0, 1)
    b = (c0 + h, L - h, 0, 1, 0, 1)
    return _split_colpat(a, max_free) + _split_colpat(b, max_free)


def drain_split(stages_pp, max_free=288):
    out = []
    for (p, k, cmp_ops, cp_ops) in stages_pp:
        nc_ops = []
        for (r0, nr, dr, colpat, colB0) in cmp_ops:
            for (_, pat) in _split_colpat(colpat, max_free):
                nb0 = colB0 + (pat[0] - colpat[0])
                nc_ops.append((r0, nr, dr, pat, nb0))
        ncp_ops = []
        for (r0, nr, pat, old) in cp_ops:
            for (_, p2) in _split_colpat(pat, max_free):
                ncp_ops.append((r0, nr, p2, old))
        out.append((p, k, nc_ops, ncp_ops))
    return out


def gen_pingpong(n, nrows, ncols, p_min=1, max_free=288):
    global N, NROWS, NCOLS
    oldN, oldR, oldC = N, NROWS, NCOLS
    N, NROWS, NCOLS = n, nrows, ncols
    try:
        full = emit_pingpong()
        filt = [(p, k, c, cp) for (p, k, c, cp) in full if p >= p_min]
        return drain_split(filt, max_free)
    finally:
        N, NROWS, NCOLS = oldN, oldR, oldC


import bass_rust
import concourse.bacc as bacc
import concourse.mybir as mybir
from concourse import tile
from concourse.bass_utils import run_bass_kernel_spmd


B, D, H1, H2, F = 2048, 3072, 512, 256, 100
NCORES = 8
BS = B // NCORES            # 256 rows per core
LEAK = 0.2
P = 128
FL = 13                     # features per core (8*13 = 104 >= 100)
FPAD = NCORES * FL          # 104
NR, NC = 4, 512
RC = 2.0 ** 23              # rounding constant
QLEV = 8190.0
MRANGE = 16.0
QSCALE = QLEV / (2 * MRANGE)
DQ = (2 * MRANGE) / QLEV

f32 = mybir.dt.float32
f16 = mybir.dt.float16
bf16 = mybir.dt.bfloat16
i16 = mybir.dt.int16
i32 = mybir.dt.int32
AF = mybir.ActivationFunctionType
ALU = mybir.AluOpType

KD, K1, K2 = D // P, H1 // P, H2 // P     # 24, 4, 2
CHUNKS = [7, 7, 7, 3]                      # uneven DMA chunks for W1/x


def sap(t_ap, pitch, pstart, pcount, coff, colpat):
    """Strided AP view: partitions [pstart, pstart+pcount), free pattern
    colpat=(c0,L,s1,c1,s2,c2) shifted to coff."""
    (c0, L, s1, c1, s2, c2) = colpat
    dims = [(pitch, pcount)]
    if c2 > 1:
        dims.append((s2, c2))
    if c1 > 1:
        dims.append((s1, c1))
    dims.append((1, L))
    a = t_ap.copy()
    a.ap = bass_rust.VecI64Pair(dims)
    a.offset = pstart * pitch + coff
    return a


SRC_OPS = gen_pingpong(256, 1, 256)
MRG_OPS = gen_pingpong(2048, 4, 512, p_min=256, max_free=512)


def emit_sort(nc, ops_table, rowpart, bufs, pitch,
              cp_engines, mir_pool=None):
    """Rotating 3-buffer odd-even merge sort (DVE min/max). The 3-buffer
    rotation avoids WAR stalls between consecutive stages. Cross-row compares
    read the B operand through an SBUF mirror copied by ACT/Pool
    (partition-shifted copies are legal in both directions)."""
    nb = len(bufs)
    ci = 0
    mi = 0
    for si, (p, k, cmp_ops, cp_ops) in enumerate(ops_table):
        cur = bufs[si % nb]
        nxt = bufs[(si + 1) % nb]
        for (r0, nr, pat, old) in cp_ops:
            pa = rowpart * r0
            npart = rowpart * nr
            src = bufs[(si - 1) % nb] if (old and si > 0) else cur
            c_in = sap(src, pitch, pa, npart, pat[0], pat)
            c_out = sap(nxt, pitch, pa, npart, pat[0], pat)
            if old and si > 0:
                cp_engines[ci % len(cp_engines)](c_out, c_in)
                ci += 1
            else:
                # fresh cells: keep the copy on DVE so the stage chain
                # stays on-engine (ACT/Pool copies add ~300ns latency)
                nc.vector.tensor_copy(c_out, c_in)
        for (r0, nr, dr, colpat, colB0) in cmp_ops:
            pa, pb = rowpart * r0, rowpart * (r0 + dr)
            npart = rowpart * nr
            a_in = sap(cur, pitch, pa, npart, colpat[0], colpat)
            a_out = sap(nxt, pitch, pa, npart, colpat[0], colpat)
            b_out = sap(nxt, pitch, pb, npart, colB0, colpat)
            if dr == 0:
                b_in = sap(cur, pitch, pb, npart, colB0, colpat)
            else:
                b_cur = sap(cur, pitch, pb, npart, colB0, colpat)
                mt = mir_pool.tile([128, 512], f32, tag="mir", bufs=6,
                                   name="mirt")
                b_in = sap(mt[:], mt[:].ap[0][0], pa, npart,
                           colpat[0], colpat)
                if mi % 2 == 0:
                    nc.scalar.copy(b_in, b_cur)
                else:
                    nc.gpsimd.tensor_copy(b_in, b_cur)
                mi += 1
            nc.vector.tensor_tensor(a_out, a_in, b_in, ALU.min)
            nc.vector.tensor_tensor(b_out, a_in, b_in, ALU.max)


def build_program():
    nc = bacc.Bacc(
        "TRN2", target_bir_lowering=False, debug=False, num_devices=NCORES)

    SM = 521                       # packed smalls: iota|lmA|lmB|b1|b2|w16|pmask|bf
    WPK = K1 * H2 + K2 * F + K2    # packed W2|T|Wfh

    xTp = nc.dram_tensor("xTp", [P, KD * BS], bf16, kind="ExternalInput").ap()
    W1p = nc.dram_tensor("W1p", [P, KD * H1], bf16, kind="ExternalInput").ap()
    wpkd = nc.dram_tensor("wpk", [P, WPK], bf16, kind="ExternalInput").ap()
    smd = nc.dram_tensor("sm", [P, SM], f32, kind="ExternalInput").ap()
    outc = nc.dram_tensor("outc", [1, B + BS], f32, kind="ExternalOutput").ap()

    with tile.TileContext(nc) as tc:
        with (
            tc.tile_pool(name="persist", bufs=1) as pers,
            tc.tile_pool(name="dram", bufs=1, space="DRAM") as dpool,
        ):
            # ---- packed persistent tile + views ----
            sm_sb = pers.tile([P, SM], f32)
            w16_v = sm_sb[:].bitcast(f16)[:, 2 * 518:2 * 518 + 1]
            pmask_v = sm_sb[:].bitcast(i16)[:, 2 * 519:2 * 519 + 1]
            bq_sb = pers.tile([P, 1], f32)
            nc.vector.memset(bq_sb[:], MRANGE * QSCALE)

            hWf_sb = pers.tile([1, BS], f32)

            # ======== phase 1: MLP (bf16 weights/activations) ========
            with (
                tc.tile_pool(name="mlp", bufs=1) as mp,
                tc.tile_pool(name="psum_mm", bufs=1, space="PSUM") as pmm,
            ):
                wpk_sb = mp.tile([P, WPK], bf16)

                xT_sb = mp.tile([P, KD * BS], bf16)
                W1_sb = mp.tile([P, KD * H1], bf16)
                lo = 0
                for c, kch in enumerate(CHUNKS):
                    nc.sync.dma_start(
                        W1_sb[:, lo * H1:(lo + kch) * H1],
                        W1p[:, lo * H1:(lo + kch) * H1])
                    nc.sync.dma_start(
                        xT_sb[:, lo * BS:(lo + kch) * BS],
                        xTp[:, lo * BS:(lo + kch) * BS])
                    lo += kch
                    if c == 0:
                        nc.sync.dma_start(sm_sb[:], smd)
                        nc.sync.dma_start(wpk_sb[:], wpkd)

                pt1 = [pmm.tile([P, BS], f32, name=f"pt1_{mb}")
                       for mb in range(K1)]
                pt2 = [pmm.tile([P, BS], f32, name=f"pt2_{mb}")
                       for mb in range(K2)]
                h1T = [mp.tile([P, BS], bf16, name=f"h1T{m}") for m in range(K1)]
                h2T = [mp.tile([P, BS], bf16, name=f"h2T{m}") for m in range(K2)]
                KLAST = KD - CHUNKS[-1]
                for k in range(KLAST):
                    for mb in range(K1):
                        nc.tensor.matmul(
                            pt1[mb][:],
                            W1_sb[:, k * H1 + mb * P: k * H1 + (mb + 1) * P],
                            xT_sb[:, k * BS:(k + 1) * BS],
                            start=(k == 0), stop=False)
                # last chunk mb-outer: each h1 block completes early so the
                # bias/leaky/W2 ladder pipelines behind the remaining matmuls
                for mb in range(K1):
                    for k in range(KLAST, KD):
                        nc.tensor.matmul(
                            pt1[mb][:],
                            W1_sb[:, k * H1 + mb * P: k * H1 + (mb + 1) * P],
                            xT_sb[:, k * BS:(k + 1) * BS],
                            start=False, stop=(k == KD - 1))
                    s1 = mp.tile([P, BS], f32, tag="stmp", bufs=2,
                                 name=f"s1_{mb}")
                    nc.scalar.activation(
                        s1[:], pt1[mb][:], AF.Identity,
                        bias=sm_sb[:, 512 + mb:513 + mb])
                    nc.vector.scalar_tensor_tensor(
                        h1T[mb][:], s1[:], LEAK, s1[:], op0=ALU.mult,
                        op1=ALU.max)
                    for mb2 in range(K2):
                        nc.tensor.matmul(
                            pt2[mb2][:],
                            wpk_sb[:, mb * H2 + mb2 * P: mb * H2 + (mb2 + 1) * P],
                            h1T[mb][:],
                            start=(mb == 0), stop=(mb == K1 - 1))
                for mb in range(K2):
                    s2 = mp.tile([P, BS], f32, tag="stmp", bufs=2,
                                 name=f"s2_{mb}")
                    nc.scalar.activation(
                        s2[:], pt2[mb][:], AF.Identity,
                        bias=sm_sb[:, 516 + mb:517 + mb])
                    nc.vector.scalar_tensor_tensor(
                        h2T[mb][:], s2[:], LEAK, s2[:], op0=ALU.mult,
                        op1=ALU.max)

                pt_m = pmm.tile([F, BS], f32, name="ptm")
                for k in range(K2):
                    nc.tensor.matmul(
                        pt_m[:], wpk_sb[:, K1 * H2 + k * F:K1 * H2 + (k + 1) * F],
                        h2T[k][:],
                        start=(k == 0), stop=(k == K2 - 1))

                ph = pmm.tile([1, BS], f32, name="ph")
                for k in range(K2):
                    nc.tensor.matmul(
                        ph[:], wpk_sb[:, K1 * H2 + K2 * F + k:K1 * H2 + K2 * F + k + 1],
                        h2T[k][:],
                        start=(k == 0), stop=(k == K2 - 1))
                nc.vector.tensor_copy(hWf_sb[:], ph[:])

                # ---- quantize + pack straight from PSUM ----
                skey = pers.tile([P, BS], f32)
                nc.scalar.activation(
                    skey[:F, :], pt_m[:], AF.Identity, bias=bq_sb[:F, :],
                    scale=QSCALE)
            sktmp = pers.tile([P, BS], f32)
            sktmp2 = pers.tile([P, BS], f32)
            nc.vector.tensor_scalar(
                skey[:F, :], skey[:F, :], scalar1=RC, scalar2=RC,
                op0=ALU.add, op1=ALU.subtract)
            nc.vector.tensor_scalar(
                skey[:F, :], skey[:F, :], scalar1=8191.0, scalar2=0.0,
                op0=ALU.min, op1=ALU.max)
            nc.vector.tensor_tensor(skey[:F, :], skey[:F, :],
                                    sm_sb[:F, 0:BS], ALU.add)
            spitch = skey[:].ap[0][0]
            emit_sort(nc, SRC_OPS, P, [skey[:], sktmp[:], sktmp2[:]], spitch,
                      [lambda o, i: nc.scalar.copy(o, i),
                       lambda o, i: nc.gpsimd.tensor_copy(o, i)])

            # ======== phase 3: AllToAll ========
            a2a_in = dpool.tile([FPAD, BS], f32)
            a2a_out = dpool.tile([FPAD, BS], f32)
            nc.sync.dma_start(a2a_in[:F, :], skey[:F, :])
            nc.sync.dma_start(a2a_in[F:FPAD, :], skey[:FPAD - F, :])
            nc.gpsimd.collective_compute(
                "AllToAll", ALU.bypass,
                replica_groups=[list(range(NCORES))],
                ins=[a2a_in.opt()], outs=[a2a_out.opt()])

            key = pers.tile([P, NC], f32)
            nc.vector.memset(key[:], 0.0)
            # fancy-AP DMAs (one per quadrant row): a2a_out rows (2r+h)*13+f,
            # col i -> key[32r+f, h*256+i]
            kpitch = key[:].ap[0][0]
            for r in range(4):
                kdst = key[:].copy()
                kdst.ap = bass_rust.VecI64Pair([(kpitch, FL), (1, 2 * BS)])
                kdst.offset = (32 * r) * kpitch
                ksrc = a2a_out[:, :].copy()
                ksrc.ap = bass_rust.VecI64Pair(
                    [(BS, FL), (FL * BS, 2), (1, BS)])
                ksrc.offset = r * 2 * FL * BS
                nc.sync.dma_start(kdst, ksrc)

            # ======== phase 4: merge (30 stages) + scan + unsort ========
            pitch = key[:].ap[0][0]
            with (
                tc.tile_pool(name="sortp", bufs=1) as sp,
                tc.tile_pool(name="psum2", bufs=1, space="PSUM") as pp2,
            ):
                tmp = sp.tile([P, NC], f32)
                tmp2 = sp.tile([P, NC], f32)
                emit_sort(nc, MRG_OPS, 32, [key[:], tmp[:], tmp2[:]], pitch,
                          cp_engines=[lambda o, i: nc.scalar.copy(o, i),
                                      lambda o, i: nc.gpsimd.tensor_copy(o, i)],
                          mir_pool=sp)

                # ---- scan phase: split key = g + j/2048 ----
                kq = sp.tile([P, NC], f32)
                nc.vector.tensor_scalar_mul(kq[:], key[:], 2048.0)
                ki = sp.tile([P, NC], i32)
                nc.vector.tensor_copy(ki[:], kq[:])
                ji = sp.tile([P, NC], i32)
                nc.vector.tensor_scalar(
                    ji[:], ki[:], scalar1=2047, scalar2=None,
                    op0=ALU.bitwise_and)
                ji16 = sp.tile([P, NC], i16)
                nc.gpsimd.tensor_copy(ji16[:], ji[:])
                # ---- unsort index prep (overlaps ACT exps below) ----
                tt = sp.tile([P, NC], i16)
                nc.vector.tensor_scalar(
                    tt[:], ji16[:], scalar1=pmask_v, scalar2=None,
                    op0=ALU.bitwise_or)
                neg1 = sp.tile([P, NC], i16)
                nc.vector.memset(neg1[:], -1)
                m0 = sp.tile([P, NC], i16)
                nc.vector.tensor_scalar(
                    m0[:], tt[:], scalar1=1023, scalar2=None, op0=ALU.is_le)
                idx0 = sp.tile([P, NC], i16)
                nc.vector.select(idx0[:], m0[:], tt[:], neg1[:])
                t1 = sp.tile([P, NC], i16)
                nc.gpsimd.tensor_scalar(
                    t1[:], tt[:], scalar1=1024, scalar2=None, op0=ALU.subtract)
                idx1 = sp.tile([P, NC], i16)
                nc.vector.select(idx1[:], m0[:], neg1[:], t1[:])
                # u/v straight from the sorted key: the iota fraction adds a
                # <= exp(DQ) ~ 1.0039 multiplicative error to each term,
                # well inside the error budget, and lets the ACT exps run in
                # parallel with the integer index chain above.
                bneg = sp.tile([P, 1], f32)
                nc.vector.memset(bneg[:], -MRANGE)
                bpos = sp.tile([P, 1], f32)
                nc.vector.memset(bpos[:], MRANGE)
                u = sp.tile([P, NC], f32)
                nc.scalar.activation(
                    u[:], key[:], AF.Exp, bias=bneg[:], scale=DQ)
                v = sp.tile([P, NC], f32)
                nc.scalar.activation(
                    v[:], key[:], AF.Exp, bias=bpos[:], scale=-DQ)

                su = sp.tile([P, NC], f32)
                nc.vector.tensor_tensor_scan(
                    su[:], u[:], u[:], initial=0.0, op0=ALU.add,
                    op1=ALU.bypass)
                sv = sp.tile([P, NC], f32)
                nc.vector.tensor_tensor_scan(
                    sv[:, NC - 1::-1], v[:, NC - 1::-1], v[:, NC - 1::-1],
                    initial=0.0, op0=ALU.add, op1=ALU.bypass)

                # cross-quadrant carries via masked prefix matmuls
                pcu = pp2.tile([P, 2], f32, name="pcu")
                nc.tensor.matmul(pcu[:, 0:1], sm_sb[:, BS:BS + P], su[:, NC - 1:NC],
                                 start=True, stop=True)
                nc.tensor.matmul(pcu[:, 1:2], sm_sb[:, BS + P:BS + 2 * P], sv[:, 0:1],
                                 start=True, stop=True)
                carr = sp.tile([P, 2], f32)
                nc.vector.tensor_copy(carr[:], pcu[:])

                s1u = sp.tile([P, NC], f32)
                nc.gpsimd.tensor_scalar(
                    s1u[:], su[:], scalar1=carr[:, 0:1], scalar2=None,
                    op0=ALU.add)
                s2vi = sp.tile([P, NC], f32)
                nc.vector.scalar_tensor_tensor(
                    s2vi[:], sv[:], carr[:, 1:2], v[:],
                    op0=ALU.add, op1=ALU.subtract)

                fa = sp.tile([P, NC], f32)
                nc.vector.tensor_tensor(fa[:], v[:], s1u[:], ALU.mult)
                fb = sp.tile([P, NC], f32)
                nc.gpsimd.tensor_tensor(fb[:], u[:], s2vi[:], ALU.mult)
                feats16 = sp.tile([P, NC], f16)
                nc.vector.tensor_tensor(feats16[:], fa[:], fb[:], ALU.add)

                # ---- unsort via local_scatter (j < 1024 | j >= 1024) ----
                dst0 = sp.tile([P, 2 * NC], f16)
                dst1 = sp.tile([P, 2 * NC], f16)
                nc.gpsimd.local_scatter(
                    dst0[:], feats16[:], idx0[:], channels=P,
                    num_elems=2 * NC, num_idxs=NC)
                nc.gpsimd.local_scatter(
                    dst1[:], feats16[:], idx1[:], channels=P,
                    num_elems=2 * NC, num_idxs=NC)

                octile = sp.tile([1, B + BS], f32)
                nc.vector.tensor_scalar(
                    octile[:, B:B + BS], hWf_sb[:],
                    scalar1=sm_sb[0:1, 520:521], scalar2=None, op0=ALU.add)
                for h, dst in ((0, dst0), (1, dst1)):
                    for s in range(2):
                        pc = pp2.tile([1, NC], f32, tag="pc", bufs=2,
                                      name=f"pc{h}{s}")
                        nc.tensor.matmul(
                            pc[:], w16_v, dst[:, s * NC:(s + 1) * NC],
                            start=True, stop=True)
                        oc_sl = octile[:, h * 1024 + s * NC:
                                       h * 1024 + (s + 1) * NC]
                        if s == 0:
                            nc.vector.tensor_copy(oc_sl, pc[:])
                        else:
                            nc.scalar.copy(oc_sl, pc[:])
                nc.sync.dma_start(outc[:], octile[:])

    nc.compile()
    return nc


def _build_in_maps(inputs):
    x = np.asarray(inputs["x"], np.float32)
    W1 = np.asarray(inputs["W1"], np.float32)
    b1 = np.asarray(inputs["b1"], np.float32)
    W2 = np.asarray(inputs["W2"], np.float32)
    b2 = np.asarray(inputs["b2"], np.float32)
    T = np.asarray(inputs["T"], np.float32)
    Wf = np.asarray(inputs["Wf"], np.float32)
    bf = np.asarray(inputs["bf"], np.float32)

    bfl = ml_dtypes.bfloat16
    W1p = np.ascontiguousarray(
        W1.reshape(KD, P, H1).transpose(1, 0, 2).reshape(P, KD * H1)
    ).astype(bfl)
    W2p = W2.reshape(K1, P, H2).transpose(1, 0, 2).reshape(P, K1 * H2)
    Tp = T.reshape(K2, P, F).transpose(1, 0, 2).reshape(P, K2 * F)
    Wfhp = Wf[:H2].reshape(K2, P).T
    wpk = np.ascontiguousarray(
        np.concatenate([W2p, Tp, Wfhp], axis=1)).astype(bfl)
    b1p = b1.reshape(K1, P).T
    b2p = b2.reshape(K2, P).T

    wff = Wf[H2:, 0]
    wff_pad = np.zeros(FPAD, np.float32)
    wff_pad[:F] = wff

    lmaskA = np.zeros((P, P), np.float32)
    lmaskB = np.zeros((P, P), np.float32)
    for k in range(P):
        for m in range(P):
            if k % 32 == m % 32:
                if k // 32 < m // 32:
                    lmaskA[k, m] = 1.0
                elif k // 32 > m // 32:
                    lmaskB[k, m] = 1.0

    pmask = np.full(P, -1, np.int16)
    for r in range(NR):
        pmask[32 * r:32 * r + FL] = 0

    in_maps = []
    for d in range(NCORES):
        m = {"W1p": W1p, "wpk": wpk}
        xT = x[d * BS:(d + 1) * BS, :].T
        m["xTp"] = np.ascontiguousarray(
            xT.reshape(KD, P, BS).transpose(1, 0, 2).reshape(P, KD * BS)
        ).astype(bfl)
        w16 = np.zeros(P, np.float16)
        for r in range(NR):
            w16[32 * r:32 * r + FL] = wff_pad[d * FL:(d + 1) * FL]
        sm = np.zeros((P, 521), np.float32)
        sm[:, 0:BS] = (d * BS + np.arange(BS, dtype=np.float32)) / 2048.0
        sm[:, BS:BS + P] = lmaskA
        sm[:, BS + P:BS + 2 * P] = lmaskB
        sm[:, 512:512 + K1] = b1p
        sm[:, 516:516 + K2] = b2p
        smv = sm.view(np.uint16)
        smv[:, 2 * 518] = w16.view(np.uint16)
        smv[:, 2 * 519] = pmask.view(np.uint16)
        sm[0, 520] = bf[0]
        m["sm"] = sm
        in_maps.append(m)
    return in_maps


_NC_CACHE = None


def _get_program():
    global _NC_CACHE
    if _NC_CACHE is None:
        _NC_CACHE = build_program()
    return _NC_CACHE


def kernel(x, W1, b1, W2, b2, T, Wf, bf):
    nc = _get_program()
    in_maps = _build_in_maps(dict(
        x=x, W1=W1, b1=b1, W2=W2, b2=b2, T=T, Wf=Wf, bf=bf))
    res = run_bass_kernel_spmd(nc, in_maps, core_ids=list(range(NCORES)))
    total = np.zeros(B, np.float64)
    for d in range(NCORES):
        oc = res.results[d]["outc"].ravel()
        total += oc[:B].astype(np.float64)
        total[d * BS:(d + 1) * BS] += oc[B:]
    return total.reshape(B, 1).astype(np.float32)
